# revision 47
# baseline (speedup 1.0000x reference)
"""Multi-head causal attention (B=2, T=2048, D=1024, H=16, hd=64) on 8 trn2 cores.

Sharding: core c handles batch b = c // 4 and the 4 heads [4*(c%4), 4*(c%4)+4).
Each core computes its partial out-projection (its heads' rows of Wo); the host
sums the 4 partials per batch and adds the bias.

Device-side math per core (all matmuls bf16 in / fp32 accumulate):
  V[2048, 256] computed directly in k-major layout (lhsT = x^T chunk), with a
  ones column appended -> V_aug; Q^T/K^T [256, 2048] computed d-major
  S^T[k, q] = K Q^T per 128k x 512q block  (2 heads row-packed in the PE array,
                                            1/sqrt(64) folded into Wq on host)
  P^T = exp(S^T - 5)  (no max subtraction: |scores| < ~6 for this data scale;
                       the constant bias cancels in the softmax ratio)
  causal: blocks fully above the diagonal skipped; blocks straddling it are
          trimmed to the valid q-range and their diagonal 128x128 square is
          multiplied by a 0/1 lower-tri mask after exp
  ctx^T[65, q] += V_aug[k, 65].T @ P^T     (row 64 = softmax denominator)
  ctx copied raw to SBUF (frees the PSUM accumulator fast), then normalized by
  reciprocal(row 64) broadcast across partitions (gpsimd)
  out[q, 1024] = ctx^T.T @ Wo_slice        (partial; host reduces). Head-pair
  1's out-projection interleaves with its attention so the output DMA overlaps.
"""

import sys

if "/opt/trn_rl_repo" not in sys.path:
    sys.path.insert(0, "/opt/trn_rl_repo")

import numpy as np
import ml_dtypes

B = 2
T = 2048
D = 1024
NH = 16
HD = 64
N_CORES = 8
P = 128
DK = D // P          # 8 contraction chunks for the projections
HPC = 4              # heads per core
DH = HPC * HD        # 256 out-dims per core per projection
TQ = 512             # q-block width
NQB = T // TQ        # 4
NKC = T // P         # 16 k-chunks
BF16 = ml_dtypes.bfloat16

_CACHE = {}


def _build_bass(n_iters=1):
    import concourse.bacc as bacc
    import concourse.tile as tile
    import concourse.mybir as mybir
    from contextlib import ExitStack

    f32 = mybir.dt.float32
    bf16 = mybir.dt.bfloat16

    nc = bacc.Bacc("TRN2", target_bir_lowering=False, debug=False,
                   num_devices=N_CORES)

    xt_d = nc.dram_tensor("xt", [P, DK, T], bf16, kind="ExternalInput")
    wq_d = nc.dram_tensor("wq", [P, DK, DH], bf16, kind="ExternalInput")
    wk_d = nc.dram_tensor("wk", [P, DK, DH], bf16, kind="ExternalInput")
    wv_d = nc.dram_tensor("wv", [P, DK, DH], bf16, kind="ExternalInput")
    wo_d = nc.dram_tensor("wo", [P, 2, D], bf16, kind="ExternalInput")
    mask_d = nc.dram_tensor("mask", [P, P], bf16, kind="ExternalInput")
    out_d = nc.dram_tensor("out", [T, D], f32, kind="ExternalOutput")

    Exp = mybir.ActivationFunctionType.Exp
    mult = mybir.AluOpType.mult

    with tile.TileContext(nc) as tc, ExitStack() as ctx:
        const = ctx.enter_context(tc.tile_pool(name="const", bufs=1))
        persist = ctx.enter_context(tc.tile_pool(name="persist", bufs=1))

        xts = const.tile([P, DK, T], bf16, tag="xts")
        wqs = const.tile([P, DK, DH], bf16, tag="wqs")
        wks = const.tile([P, DK, DH], bf16, tag="wks")
        wvs = const.tile([P, DK, DH], bf16, tag="wvs")
        wos = const.tile([P, 2, D], bf16, tag="wos")
        mask = const.tile([P, P], bf16, tag="mask")

        # Wv first (gates the V matmuls), then the first halves of every x
        # chunk (the V/projection matmuls consume chunks in kc order), then
        # the rest. Half-chunk granularity lets the first matmuls start
        # several us earlier.
        H2 = T // 2
        nc.sync.dma_start(wvs[:], wv_d.ap())
        for kc in range(DK):
            nc.sync.dma_start(xts[:, kc, :H2], xt_d.ap()[:, kc, :H2])
        nc.sync.dma_start(wks[:], wk_d.ap())
        nc.sync.dma_start(wqs[:], wq_d.ap())
        nc.sync.dma_start(mask[:], mask_d.ap())
        for kc in range(DK):
            nc.sync.dma_start(xts[:, kc, H2:], xt_d.ap()[:, kc, H2:])
        nc.sync.dma_start(wos[:], wo_d.ap())

        # per head-pair chunk m: Q^T/K^T stored [128 dims, T]
        qt = [persist.tile([P, T], bf16, tag=f"qt{m}", name=f"qt{m}")
              for m in range(2)]
        kt = [persist.tile([P, T], bf16, tag=f"kt{m}", name=f"kt{m}")
              for m in range(2)]
        vaug = [persist.tile([P, NKC, 2, HD + 1], bf16, tag=f"vaug{m}",
                             name=f"vaug{m}") for m in range(2)]
        ctxT = [persist.tile([P, T], bf16, tag=f"ctxT{m}", name=f"ctxT{m}")
                for m in range(2)]

        # PSUM budget is 8 banks: proj(2) + s(4) + ctx(2) while projections
        # overlap attention; proj closes before out_ps(2) opens so the
        # out-projection interleaves with head-pair 1's attention.
        s_ps = ctx.enter_context(tc.tile_pool(name="s_ps", bufs=2,
                                              space="PSUM"))
        ctx_ps = ctx.enter_context(tc.tile_pool(name="ctx_ps", bufs=2,
                                                space="PSUM"))
        pt_pool = ctx.enter_context(tc.tile_pool(name="pt", bufs=8))
        sm_pool = ctx.enter_context(tc.tile_pool(name="small", bufs=12))
        cs_pool = ctx.enter_context(tc.tile_pool(name="ctxsb", bufs=8))
        # opened last of the PSUM pools so it can close first (LIFO), freeing
        # its 2 banks for out_ps during head-pair 1's attention
        proj_ctx = ExitStack()
        proj_ps = proj_ctx.enter_context(
            tc.tile_pool(name="proj_ps", bufs=2, space="PSUM"))
        _env = {}

        def _proj_tile():
            return _env["proj_ps"].tile([P, TQ], f32, tag="proj", name="projps")

        # warm the exp table set during the input-DMA window; nbias is the
        # constant subtracted inside exp (cancels in the softmax ratio)
        warm = sm_pool.tile([1, 16], f32, tag="warm", name="warm")
        nc.vector.memset(warm[:], 0.0)
        nc.scalar.activation(warm[:], warm[:], Exp)
        nbias = const.tile([P, 1], f32, tag="nbias")
        nc.vector.memset(nbias[:], -5.0)

        def v_direct(rcs):
            # V in natural (k-major) layout: one [128 rows, 256 hd] block per
            # row chunk, lhsT = x^T slice, rhs = Wv (all 4 heads at once)
            for rc in rcs:
                ps = _proj_tile()
                for kc in range(DK):
                    nc.tensor.matmul(
                        ps[:, :DH],
                        lhsT=xts[:, kc, rc * P:(rc + 1) * P],
                        rhs=wvs[:, kc, :],
                        start=(kc == 0), stop=(kc == DK - 1))
                for m in range(2):
                    nc.vector.tensor_copy(
                        out=vaug[m][:, rc, :, :HD],
                        in_=ps[:, m * P:(m + 1) * P].rearrange(
                            "p (h d) -> p h d", h=2))

        def projections(m, nbs=range(NQB)):
            # K then Q per n-block, so attention q-block nb can start right
            # after its own projections instead of after all of K
            for nb in nbs:
                for w_sb, dst in ((wks, kt[m]), (wqs, qt[m])):
                    ps = _proj_tile()
                    for kc in range(DK):
                        nc.tensor.matmul(
                            ps[:],
                            lhsT=w_sb[:, kc, m * P:(m + 1) * P],
                            rhs=xts[:, kc, nb * TQ:(nb + 1) * TQ],
                            start=(kc == 0), stop=(kc == DK - 1))
                    nc.vector.tensor_copy(
                        out=dst[:, nb * TQ:(nb + 1) * TQ], in_=ps[:])

        def attention_qb(hp, qb, last=False, mid=None):
            nkc = 4 * qb + 4
            ctxp = [ctx_ps.tile([HD + 1, TQ], f32, tag="ctx", name=f"ctx{j}")
                    for j in range(2)]
            # software-pipelined: emit scores(kc) before ctx(kc-1) so the PE
            # stream never head-of-line blocks on the exp the ctx matmuls wait
            # for (engine streams execute in order)
            pend = None

            def ctx_mms(kc, off):
                for j in range(2):
                    nc.tensor.matmul(
                        ctxp[j][:, off:],
                        lhsT=vaug[hp][:, kc, j, :],
                        rhs=pts[kc][:, j, off:],
                        start=(kc == 0), stop=(kc == nkc - 1),
                        skip_group_check=True)

            pts = {}
            for kc in range(nkc):
                d = kc - 4 * qb
                off = P * d if d > 0 else 0
                sp = s_ps.tile([P, 2, TQ], f32, tag="sp")
                for j in range(2):
                    nc.tensor.matmul(
                        sp[:, j, off:],
                        lhsT=kt[hp][64 * j:64 * j + 64, kc * P:(kc + 1) * P],
                        rhs=qt[hp][64 * j:64 * j + 64,
                                   qb * TQ + off:(qb + 1) * TQ],
                        start=True, stop=True)
                pt = pt_pool.tile([P, 2, TQ], bf16, tag="pt")
                pts[kc] = pt
                # constant bias inside exp cancels in the softmax ratio but
                # guards against overflow for any plausible score scale
                nc.scalar.activation(pt[:, :, off:], sp[:, :, off:], Exp,
                                     bias=nbias[:])
                if d >= 0:
                    diag = pt[:, :, off:off + P]
                    nc.vector.tensor_tensor(
                        diag, diag,
                        mask[:, None, :].to_broadcast([P, 2, P]), mult)
                if pend is not None:
                    ctx_mms(*pend)
                pend = (kc, off)
                if mid is not None and kc == nkc // 2:
                    mid()
            ctx_mms(*pend)
            for j in (1, 0):
                # raw copy releases the PSUM accumulator; normalization happens
                # from SBUF off the critical path. For the final q-block
                # nothing waits on the accumulator, so normalize straight from
                # PSUM and skip the copy (shorter tail latency).
                if last:
                    craw = ctxp[j]
                else:
                    craw = cs_pool.tile([HD + 1, TQ], f32, tag="craw")
                    nc.vector.tensor_copy(out=craw[:], in_=ctxp[j][:])
                rec = sm_pool.tile([1, TQ], f32, tag="rec")
                nc.vector.reciprocal(rec[:], craw[HD:HD + 1, :])
                bc = sm_pool.tile([HD, TQ], f32, tag="bc")
                nc.gpsimd.partition_broadcast(bc[:], rec[:], channels=HD)
                # for the final q-block, normalize in 128-column chunks so
                # the out-projection can start on the first chunk while the
                # rest are still being normalized
                for ch in (range(4) if last else (None,)):
                    cs_ = (slice(None) if ch is None
                           else slice(ch * P, (ch + 1) * P))
                    w0 = qb * TQ + (0 if ch is None else ch * P)
                    w1 = (qb + 1) * TQ if ch is None else w0 + P
                    if j == 0:
                        nc.vector.tensor_tensor(
                            ctxT[hp][0:HD, w0:w1],
                            craw[0:HD, cs_], bc[:, cs_], mult)
                    else:
                        tmp = sm_pool.tile([HD, TQ], bf16, tag="ctmp",
                                           name="ctmp")
                        nc.vector.tensor_tensor(
                            tmp[:, cs_], craw[0:HD, cs_], bc[:, cs_], mult)
                        nc.sync.dma_start(
                            ctxT[hp][HD:P, w0:w1], tmp[:, cs_])

        def out_proj_qb(qb, out_ps, ob_pool, split_dma=False,
                        alt_act=False):
            for rc in range(4 * qb, 4 * qb + 4):
                ob = ob_pool.tile([P, D], f32, tag="ob")
                for nb2 in range(2):
                    op = out_ps.tile([P, TQ], f32, tag="op")
                    for hc in range(2):
                        nc.tensor.matmul(
                            op[:],
                            lhsT=ctxT[hc][:, rc * P:(rc + 1) * P],
                            rhs=wos[:, hc, nb2 * TQ:(nb2 + 1) * TQ],
                            start=(hc == 0), stop=(hc == 1))
                    # in the kernel tail ACT is idle: split the copies across
                    # both engines so they drain in parallel
                    if alt_act and nb2 == 1:
                        nc.scalar.copy(
                            out=ob[:, nb2 * TQ:(nb2 + 1) * TQ], in_=op[:])
                    else:
                        nc.vector.tensor_copy(
                            out=ob[:, nb2 * TQ:(nb2 + 1) * TQ], in_=op[:])
                    if split_dma:
                        nc.sync.dma_start(
                            out_d.ap()[rc * P:(rc + 1) * P,
                                       nb2 * TQ:(nb2 + 1) * TQ],
                            ob[:, nb2 * TQ:(nb2 + 1) * TQ])
                if not split_dma:
                    nc.sync.dma_start(out_d.ap()[rc * P:(rc + 1) * P, :],
                                      ob[:])

        for it in range(n_iters):
            _env["proj_ps"] = proj_ps
            if it > 0:
                # re-open the projection-psum zone for benchmark iterations
                proj_ctx = ExitStack()
                proj_ps = proj_ctx.enter_context(
                    tc.tile_pool(name=f"proj_ps{it}", bufs=2, space="PSUM"))
                _env["proj_ps"] = proj_ps
            for m in range(2):
                nc.vector.memset(vaug[m][:, :, :, HD:], 1.0)
            v_direct(range(NKC))
            projections(0)
            # spread head-pair 1's projections between attention q-blocks:
            # dense PE filler sits behind each ACT-bound attention stretch
            attention_qb(0, 0)
            attention_qb(0, 1)
            projections(1, (0,))
            attention_qb(0, 2)
            projections(1, (1,))
            attention_qb(0, 3)
            projections(1, (2, 3))
            proj_ctx.close()

            with tc.tile_pool(name=f"out_ps{it}", bufs=2,
                              space="PSUM") as out_ps, \
                    tc.tile_pool(name=f"ob{it}", bufs=8) as ob_pool:
                # emit D(qb) after attention(qb+1): PE streams are in-order,
                # so this keeps the next q-block's score matmuls from being
                # head-of-line blocked behind D's wait on the epilogue chain
                attention_qb(1, 0)
                attention_qb(1, 1)
                out_proj_qb(0, out_ps, ob_pool)
                attention_qb(1, 2)
                out_proj_qb(1, out_ps, ob_pool)
                attention_qb(1, 3, last=True)
                out_proj_qb(2, out_ps, ob_pool, split_dma=True)
                out_proj_qb(NQB - 1, out_ps, ob_pool, split_dma=True)

    nc.compile()
    return nc


def get_nc(n_iters=1):
    key = ("nc", n_iters)
    if key not in _CACHE:
        _CACHE[key] = _build_bass(n_iters)
    return _CACHE[key]


def make_in_maps(x, Wq, Wk, Wv, Wo):
    """Per-core input dicts. Core c: batch c//4, heads [4*(c%4), 4*(c%4)+4)."""
    x = np.asarray(x, np.float32)
    scale = 1.0 / np.sqrt(np.float32(HD))
    mask = np.tril(np.ones((P, P), np.float32)).T  # mask[k,q] = 1 if k <= q
    mask = mask.astype(BF16)

    def fold(w):  # [D, DH] -> [P, DK, DH]
        return np.ascontiguousarray(
            w.reshape(DK, P, -1).transpose(1, 0, 2)).astype(BF16)

    in_maps = []
    for c in range(N_CORES):
        b, g = divmod(c, 4)
        cs = slice(DH * g, DH * (g + 1))
        xt = np.ascontiguousarray(
            x[b].T.reshape(DK, P, T).transpose(1, 0, 2)).astype(BF16)
        wo = np.ascontiguousarray(
            np.asarray(Wo, np.float32)[cs].reshape(2, P, D).transpose(1, 0, 2)
        ).astype(BF16)
        in_maps.append({
            "xt": xt,
            "wq": fold(np.asarray(Wq, np.float32)[:, cs] * scale),
            "wk": fold(np.asarray(Wk, np.float32)[:, cs]),
            "wv": fold(np.asarray(Wv, np.float32)[:, cs]),
            "wo": wo,
            "mask": mask,
        })
    return in_maps


def kernel(x, Wq, Wk, Wv, Wo, bo):
    from concourse import bass_utils

    nc = get_nc()
    in_maps = make_in_maps(x, Wq, Wk, Wv, Wo)
    res = bass_utils.run_bass_kernel_spmd(
        nc, in_maps, core_ids=list(range(N_CORES)))
    parts = [r["out"] for r in res.results]
    out = np.empty((B, T, D), np.float32)
    for b in range(B):
        out[b] = parts[4 * b]
        for g in range(1, 4):
            out[b] += parts[4 * b + g]
        out[b] += np.asarray(bo, np.float32)
    return out
